# revision 5
# baseline (speedup 1.0000x reference)
"""RNN-T joint network kernel for 8 Trainium2 NeuronCores.

out[b,t,u,:] = W2 @ tanh(W1e @ enc[b,t] + W1d @ dec[b,u] + b1) + b2

Shapes: B=4, T=200, U=100, D=512, H=1024, O=512 (all fp32).
Sharding: T split 8 ways (25 t's per core); dec + weights replicated.

Per-core device program:
  Phase 1: enc_hT[h, b*25+t] = W1e @ encT (+b1), dec_hT[h, b*100+u] = W1d @ decT
           (h on partitions in 8 chunks of 128; small matmuls).
  Phase 2: for each chunk (b, 5 t's) = 500 rows:
           s[kchunk, t, u] = dec_hT[k][:, b-block] (+) enc_hT broadcast  (DVE,
           stride-0 broadcast APs), tanh over the whole [128, 4000] tile (ACT),
           then 4x8 accumulating matmuls against W2T chunks -> psum [128, 512],
           add b2 + copy to SBUF (DVE), DMA out.
"""

from contextlib import ExitStack

import numpy as np

import concourse.bacc as bacc
import concourse.bass as bass
import concourse.mybir as mybir
import concourse.tile as tile
from concourse.bass_utils import run_bass_kernel_spmd

F32 = mybir.dt.float32

B, T, U, D, H, O = 4, 200, 100, 512, 1024, 512
NCORES = 8
TLOC = T // NCORES            # 25 t's per core
PAIRS = B * TLOC              # 100 (b,t) pairs per core
TCH = 5                       # t's per inner chunk
CHROWS = TCH * U              # 500 rows per chunk
NCH = TLOC // TCH             # 5 chunks per b
ROWS = PAIRS * U              # 10000 output rows per core
DK = D // 128                 # 4 contraction chunks for phase 1
HK = H // 128                 # 8 h chunks

_CACHE = {}


def _build():
    nc = bacc.Bacc("TRN2", target_bir_lowering=False, debug=False,
                   num_devices=NCORES)
    encT = nc.dram_tensor("encT", [D, PAIRS], F32, kind="ExternalInput")
    decT = nc.dram_tensor("decT", [D, B * U], F32, kind="ExternalInput")
    w1eT = nc.dram_tensor("w1eT", [D, H], F32, kind="ExternalInput")
    w1dT = nc.dram_tensor("w1dT", [D, H], F32, kind="ExternalInput")
    w2T = nc.dram_tensor("w2T", [H, O], F32, kind="ExternalInput")
    b1r = nc.dram_tensor("b1r", [128, HK], F32, kind="ExternalInput")
    b2r = nc.dram_tensor("b2r", [128, O], F32, kind="ExternalInput")
    out = nc.dram_tensor("out", [ROWS, O], F32, kind="ExternalOutput")

    BU = B * U
    with tile.TileContext(nc) as tc, ExitStack() as ctx:
        consts = ctx.enter_context(tc.tile_pool(name="consts", bufs=1))
        spool = ctx.enter_context(tc.tile_pool(name="spool", bufs=3))
        opool = ctx.enter_context(tc.tile_pool(name="opool", bufs=4))
        psA = ctx.enter_context(tc.tile_pool(name="psA", bufs=2, space="PSUM"))
        psB = ctx.enter_context(tc.tile_pool(name="psB", bufs=4, space="PSUM"))

        # ---- load constants / inputs ----
        w1e_s = consts.tile([128, DK * H], F32)      # dk-chunk k at cols [k*H, (k+1)*H)
        w1d_s = consts.tile([128, DK * H], F32)
        w2_s = consts.tile([128, HK * O], F32)       # hk-chunk k at cols [k*O, (k+1)*O)
        encT_s = consts.tile([128, DK * PAIRS], F32)
        decT_s = consts.tile([128, DK * BU], F32)
        b1_s = consts.tile([128, HK], F32)
        b2_s = consts.tile([128, O], F32)
        def load_chunked(sbuf_t, dram_t, nchunk, width):
            # dram [nchunk*128, width] -> sbuf [128, nchunk*width], chunk k at
            # cols [k*width, (k+1)*width), as a single DMA.
            src = dram_t[:].rearrange("(k p) w -> p k w", k=nchunk)
            dst = sbuf_t[:].rearrange("p (k w) -> p k w", k=nchunk)
            nc.sync.dma_start(dst, src)

        load_chunked(w1e_s, w1eT, DK, H)
        load_chunked(w1d_s, w1dT, DK, H)
        load_chunked(encT_s, encT, DK, PAIRS)
        load_chunked(decT_s, decT, DK, BU)
        load_chunked(w2_s, w2T, HK, O)
        nc.sync.dma_start(b1_s[:], b1r[:])
        nc.sync.dma_start(b2_s[:], b2r[:])

        # ---- phase 1: enc_hT (+b1) and dec_hT ----
        ench_s = consts.tile([128, HK * PAIRS], F32)  # hk-chunk k at cols [k*PAIRS, ...)
        dech_s = consts.tile([128, HK * BU], F32)
        for hk in range(HK):
            pe = psA.tile([128, PAIRS], F32, tag="psA_e")
            for dk in range(DK):
                nc.tensor.matmul(
                    pe[:],
                    lhsT=w1e_s[:, dk * H + hk * 128: dk * H + (hk + 1) * 128],
                    rhs=encT_s[:, dk * PAIRS:(dk + 1) * PAIRS],
                    start=(dk == 0), stop=(dk == DK - 1),
                )
            nc.vector.tensor_scalar_add(
                ench_s[:, hk * PAIRS:(hk + 1) * PAIRS], pe[:], b1_s[:, hk:hk + 1])
            pd = psA.tile([128, BU], F32, tag="psA_d")
            for dk in range(DK):
                nc.tensor.matmul(
                    pd[:],
                    lhsT=w1d_s[:, dk * H + hk * 128: dk * H + (hk + 1) * 128],
                    rhs=decT_s[:, dk * BU:(dk + 1) * BU],
                    start=(dk == 0), stop=(dk == DK - 1),
                )
            nc.vector.tensor_copy(dech_s[:, hk * BU:(hk + 1) * BU], pd[:])

        # ---- phase 2: chunks of (b, 5 t's) ----
        for b in range(B):
            for tc_i in range(NCH):
                s_t = spool.tile([128, HK * CHROWS], F32, tag="s")
                for k in range(HK):
                    in0 = dech_s[:, k * BU + b * U: k * BU + (b + 1) * U]
                    in0 = in0.rearrange("p (a u) -> p a u", a=1)
                    c0 = k * PAIRS + b * TLOC + tc_i * TCH
                    in1 = ench_s[:, c0:c0 + TCH].rearrange("p (t a) -> p t a", a=1)
                    bc0, bc1 = bass.broadcast_tensor_aps(in0, in1)
                    outap = s_t[:, k * CHROWS:(k + 1) * CHROWS].rearrange(
                        "p (t u) -> p t u", t=TCH)
                    nc.vector.tensor_tensor(outap, bc0, bc1, mybir.AluOpType.add)
                nc.scalar.activation(s_t[:], s_t[:], mybir.ActivationFunctionType.Tanh)
                row0 = b * (TLOC * U) + tc_i * CHROWS
                for m in range((CHROWS + 127) // 128):
                    M = min(128, CHROWS - m * 128)
                    ps = psB.tile([128, O], F32, tag="psB")
                    for k in range(HK):
                        nc.tensor.matmul(
                            ps[:M, :],
                            lhsT=s_t[:, k * CHROWS + m * 128: k * CHROWS + m * 128 + M],
                            rhs=w2_s[:, k * O:(k + 1) * O],
                            start=(k == 0), stop=(k == HK - 1),
                        )
                    ot = opool.tile([128, O], F32, tag="ot")
                    nc.vector.tensor_tensor(ot[:M, :], ps[:M, :], b2_s[:M, :],
                                            mybir.AluOpType.add)
                    r = row0 + m * 128
                    nc.sync.dma_start(out[r:r + M, :], ot[:M, :])
    nc.compile()
    return nc


def kernel(enc_state, dec_state, W1, b1, W2, b2, _trace=False):
    enc_state = np.ascontiguousarray(enc_state, dtype=np.float32)
    dec_state = np.ascontiguousarray(dec_state, dtype=np.float32)
    W1 = np.asarray(W1, dtype=np.float32)
    b1 = np.asarray(b1, dtype=np.float32)
    W2 = np.asarray(W2, dtype=np.float32)
    b2 = np.asarray(b2, dtype=np.float32)

    if "nc" not in _CACHE:
        _CACHE["nc"] = _build()
    nc = _CACHE["nc"]

    decT = np.ascontiguousarray(dec_state.reshape(B * U, D).T)          # [D, 400]
    w1eT = np.ascontiguousarray(W1[:, :D].T)                            # [D, H]
    w1dT = np.ascontiguousarray(W1[:, D:].T)                            # [D, H]
    w2T = np.ascontiguousarray(W2.T)                                    # [H, O]
    b1r = np.ascontiguousarray(b1.reshape(HK, 128).T)                   # [128, HK]
    b2r = np.ascontiguousarray(np.broadcast_to(b2, (128, O)))           # [128, O]

    in_maps = []
    for c in range(NCORES):
        enc_c = enc_state[:, c * TLOC:(c + 1) * TLOC, :].reshape(PAIRS, D)
        encT_c = np.ascontiguousarray(enc_c.T)                          # [D, 100]
        in_maps.append({
            "encT": encT_c, "decT": decT, "w1eT": w1eT, "w1dT": w1dT,
            "w2T": w2T, "b1r": b1r, "b2r": b2r,
        })

    res = run_bass_kernel_spmd(nc, in_maps, list(range(NCORES)), trace=_trace)
    out = np.empty((B, T, U, O), dtype=np.float32)
    for c in range(NCORES):
        out[:, c * TLOC:(c + 1) * TLOC] = res.results[c]["out"].reshape(B, TLOC, U, O)
    if _trace:
        kernel.last_results = res
    return out


# revision 31
# speedup vs baseline: 3.1172x; 3.1172x over previous
"""RNN-T joint network kernel for 8 Trainium2 NeuronCores.

out[b,t,u,:] = W2 @ tanh(W1e @ enc[b,t] + W1d @ dec[b,u] + b1) + b2

Shapes: B=4, T=200, U=100, D=512, H=1024, O=512 (all fp32).
Sharding: T split 8 ways (25 t's per core); dec + weights replicated.

Per-core device program:
  Phase 1: enc_hT[h, b*25+t] = W1e @ encT (+b1), dec_hT[h, b*100+u] = W1d @ decT
           (h on partitions in 8 chunks of 128; small matmuls).
  Phase 2: for each chunk (b, 5 t's) = 500 rows:
           s[kchunk, t, u] = dec_hT[k][:, b-block] (+) enc_hT broadcast  (DVE,
           stride-0 broadcast APs), tanh over the whole [128, 4000] tile (ACT),
           then 4x8 accumulating matmuls against W2T chunks -> psum [128, 512],
           add b2 + copy to SBUF (DVE), DMA out.
"""

from contextlib import ExitStack

import numpy as np

import concourse.bacc as bacc
import concourse.bass as bass
import concourse.mybir as mybir
import concourse.tile as tile
from concourse.bass_utils import run_bass_kernel_spmd

F32 = mybir.dt.float32
F32R = mybir.dt.float32r

B, T, U, D, H, O = 4, 200, 100, 512, 1024, 512
NCORES = 8
TLOC = T // NCORES            # 25 t's per core
PAIRS = B * TLOC              # 100 (b,t) pairs per core
TCH = 5                       # t's per inner chunk
CHROWS = TCH * U              # 500 rows per chunk
NCH = TLOC // TCH             # 5 chunks per b
ROWS = PAIRS * U              # 10000 output rows per core
DK = D // 128                 # 4 contraction chunks for phase 1
HK = H // 128                 # 8 h chunks

_CACHE = {}


def _build():
    nc = bacc.Bacc("TRN2", target_bir_lowering=False, debug=False,
                   num_devices=NCORES)
    # inputs arrive pre-interleaved in SBUF layout: [128, nchunk*width],
    # partition p holding chunk k's row (k*128+p) at cols [k*width, ...)
    encT = nc.dram_tensor("encT", [128, DK * PAIRS], F32, kind="ExternalInput")
    decT = nc.dram_tensor("decT", [128, DK * B * U], F32, kind="ExternalInput")
    w1eT = nc.dram_tensor("w1eT", [128, DK * H], F32, kind="ExternalInput")
    w1dT = nc.dram_tensor("w1dT", [128, DK * H], F32, kind="ExternalInput")
    w2T = nc.dram_tensor("w2T", [128, HK * O], F32, kind="ExternalInput")
    b1r = nc.dram_tensor("b1r", [128, HK], F32, kind="ExternalInput")
    b2c = nc.dram_tensor("b2c", [128, O // 128], F32, kind="ExternalInput")
    out = nc.dram_tensor("out", [O, ROWS], F32, kind="ExternalOutput")

    BU = B * U
    with tile.TileContext(nc) as tc, ExitStack() as ctx:
        consts = ctx.enter_context(tc.tile_pool(name="consts", bufs=1))
        spool = ctx.enter_context(tc.tile_pool(name="spool", bufs=3))
        opool = ctx.enter_context(tc.tile_pool(name="opool", bufs=4))
        psB = ctx.enter_context(tc.tile_pool(name="psB", bufs=8, space="PSUM"))

        # ---- load constants / inputs ----
        w1e_s = consts.tile([128, DK * H], F32)      # dk-chunk k at cols [k*H, (k+1)*H)
        w1d_s = consts.tile([128, DK * H], F32)
        w2_s = consts.tile([128, HK * O], F32)       # hk-chunk k at cols [k*O, (k+1)*O)
        encT_s = consts.tile([128, DK * PAIRS], F32)
        decT_s = consts.tile([128, DK * BU], F32)
        b1_s = consts.tile([128, HK], F32)
        b2c_s = consts.tile([128, O // 128], F32)
        # split loads across the two HWDGE rings (sync + scalar) so the
        # enc-side and dec-side transfers run in parallel; all plain 2D
        # contiguous DMAs (inputs are pre-interleaved on the host)
        nc.sync.dma_start(encT_s[:], encT[:])
        nc.scalar.dma_start(decT_s[:], decT[:])
        nc.sync.dma_start(w1e_s[:], w1eT[:])
        nc.scalar.dma_start(w1d_s[:], w1dT[:])
        nc.sync.dma_start(w2_s[:], w2T[:])
        nc.scalar.dma_start(b1_s[:], b1r[:])
        nc.scalar.dma_start(b2c_s[:], b2c[:])

        # float32r copies (fp32r matmul inputs must come from rounding
        # producers; DMA does not qualify)
        w1e_r = consts.tile([128, DK * H], F32R)
        w1d_r = consts.tile([128, DK * H], F32R)
        w2_r = consts.tile([128, HK * O], F32R)
        encT_r = consts.tile([128, DK * PAIRS], F32R)
        decT_r = consts.tile([128, DK * BU], F32R)
        nc.vector.tensor_copy(encT_r[:], encT_s[:])
        nc.vector.tensor_copy(w1e_r[:], w1e_s[:])
        nc.vector.tensor_copy(decT_r[:], decT_s[:])
        nc.vector.tensor_copy(w1d_r[:], w1d_s[:])
        nc.vector.tensor_copy(w2_r[:], w2_s[:])

        # ---- phase 1: enc_hT (+b1) and dec_hT ----
        # per-k tiles so phase-2 builds can start as soon as *their* k chunk
        # is ready (a single big tile would serialize phase 2 behind all of
        # phase 1 via coarse dependency tracking)
        ench_t = [consts.tile([128, PAIRS], F32, name=f"ench{k}") for k in range(HK)]
        dech_t = [consts.tile([128, BU], F32, name=f"dech{k}") for k in range(HK)]
        # enc matmuls first: they only need encT+w1e, and cover the
        # decT/w1d DMA + cast latency with PE work
        for hk in range(HK):
            pe = psB.tile([128, 512], F32, tag="psB", name="pe")
            pe = pe[:, :PAIRS]
            for dk in range(DK):
                nc.tensor.matmul(
                    pe[:],
                    lhsT=w1e_r[:, dk * H + hk * 128: dk * H + (hk + 1) * 128],
                    rhs=encT_r[:, dk * PAIRS:(dk + 1) * PAIRS],
                    start=(dk == 0), stop=(dk == DK - 1),
                )
            nc.vector.tensor_scalar_add(ench_t[hk][:], pe[:], b1_s[:, hk:hk + 1])
        for hk in range(HK):
            pd = psB.tile([128, 512], F32, tag="psB", name="pd")
            pd = pd[:, :BU]
            for dk in range(DK):
                nc.tensor.matmul(
                    pd[:],
                    lhsT=w1d_r[:, dk * H + hk * 128: dk * H + (hk + 1) * 128],
                    rhs=decT_r[:, dk * BU:(dk + 1) * BU],
                    start=(dk == 0), stop=(dk == DK - 1),
                )
            nc.vector.tensor_copy(dech_t[hk][:], pd[:])

        # ---- phase 2: chunks of (b, up to 5 t's) ----
        # small leading chunks shorten the build+tanh fill before the first
        # big matmul group
        chunks = []
        for b in range(B):
            sizes = [1, 4] + [TCH] * 4 if b == 0 else [TCH] * NCH
            t0c = 0
            for tch in sizes:
                chunks.append((b, t0c, tch))
                t0c += tch
        for b, t0c, tch in chunks:
            rows_c = tch * U
            s_t = spool.tile([128, HK * CHROWS], F32R, tag="s")
            for k in range(HK):
                in0 = dech_t[k][:, b * U:(b + 1) * U]
                in0 = in0.rearrange("p (a u) -> p a u", a=1)
                c0 = b * TLOC + t0c
                in1 = ench_t[k][:, c0:c0 + tch].rearrange("p (t a) -> p t a", a=1)
                bc0, bc1 = bass.broadcast_tensor_aps(in0, in1)
                outap = s_t[:, k * CHROWS: k * CHROWS + rows_c].rearrange(
                    "p (t u) -> p t u", t=tch)
                nc.vector.tensor_tensor(outap, bc0, bc1, mybir.AluOpType.add)
            s_used = s_t[:].rearrange("p (k c) -> p k c", k=HK)[:, :, :rows_c]
            nc.scalar.activation(s_used, s_used,
                                 mybir.ActivationFunctionType.Tanh)
            row0 = b * (TLOC * U) + t0c * U
            # swapped matmul: W2 blocks stationary, s moving -> psum holds
            # out^T [o-chunk, rows]; b2 folds into the psum->sbuf copy as a
            # per-partition bias.
            for oc in range(O // 128):
                ps = psB.tile([128, 512], F32, tag="psB")
                for k in range(HK):
                    nc.tensor.matmul(
                        ps[:, :rows_c],
                        lhsT=w2_r[:, k * O + oc * 128: k * O + (oc + 1) * 128],
                        rhs=s_t[:, k * CHROWS: k * CHROWS + rows_c],
                        start=(k == 0), stop=(k == HK - 1),
                    )
                ot = opool.tile([128, CHROWS], F32, tag="ot")
                if oc < 2:
                    nc.scalar.activation(
                        ot[:, :rows_c], ps[:, :rows_c],
                        mybir.ActivationFunctionType.Identity,
                        bias=b2c_s[:, oc:oc + 1])
                else:
                    nc.vector.tensor_scalar_add(
                        ot[:, :rows_c], ps[:, :rows_c], b2c_s[:, oc:oc + 1])
                nc.sync.dma_start(
                    out[oc * 128:(oc + 1) * 128, row0:row0 + rows_c],
                    ot[:, :rows_c])
    nc.compile()
    return nc


def kernel(enc_state, dec_state, W1, b1, W2, b2, _trace=False):
    enc_state = np.ascontiguousarray(enc_state, dtype=np.float32)
    dec_state = np.ascontiguousarray(dec_state, dtype=np.float32)
    W1 = np.asarray(W1, dtype=np.float32)
    b1 = np.asarray(b1, dtype=np.float32)
    W2 = np.asarray(W2, dtype=np.float32)
    b2 = np.asarray(b2, dtype=np.float32)

    if "nc" not in _CACHE:
        _CACHE["nc"] = _build()
    nc = _CACHE["nc"]

    def chunk128(a):
        # [n*128, w] -> [128, n*w]: partition p holds row k*128+p of chunk k
        n = a.shape[0] // 128
        return np.ascontiguousarray(
            a.reshape(n, 128, a.shape[1]).transpose(1, 0, 2).reshape(128, -1))

    decT = chunk128(dec_state.reshape(B * U, D).T)                      # [128, 4*400]
    w1eT = chunk128(W1[:, :D].T)                                        # [128, 4*H]
    w1dT = chunk128(W1[:, D:].T)                                        # [128, 4*H]
    w2T = chunk128(W2.T)                                                # [128, 8*O]
    b1r = np.ascontiguousarray(b1.reshape(HK, 128).T)                   # [128, HK]
    b2cm = np.ascontiguousarray(b2.reshape(O // 128, 128).T)            # [128, 4]

    in_maps = []
    for c in range(NCORES):
        enc_c = enc_state[:, c * TLOC:(c + 1) * TLOC, :].reshape(PAIRS, D)
        encT_c = chunk128(enc_c.T)                                      # [128, 4*100]
        in_maps.append({
            "encT": encT_c, "decT": decT, "w1eT": w1eT, "w1dT": w1dT,
            "w2T": w2T, "b1r": b1r, "b2c": b2cm,
        })

    res = run_bass_kernel_spmd(nc, in_maps, list(range(NCORES)), trace=_trace)
    out = np.empty((B, T, U, O), dtype=np.float32)
    for c in range(NCORES):
        # device output is transposed: [O, ROWS]
        out[:, c * TLOC:(c + 1) * TLOC] = (
            res.results[c]["out"].T.reshape(B, TLOC, U, O))
    if _trace:
        kernel.last_results = res
    return out
